# revision 3
# baseline (speedup 1.0000x reference)
"""Trainium2 Bass kernel for the masked per-position CNN contraction.

Computes, for each lattice position p (N=16384 total):
    y[p, i] = sum_{j,k} W[i,j,k] * mask[p,i,j,k] * x[j, ker[p,k]]
    out[i, p] = elu(y[p, i] + b[i])

Strategy (8 NeuronCores, position axis sharded 2048/core):
  - mask (151 MB fp32) dominates traffic -> memory-bound. Host transposes the
    per-core mask slice to a [(i,j,k), p] layout whose 2304-long contraction
    axis is split into 18 partition-aligned chunks of 128 (k=9 is split 8+1 so
    chunk boundaries align with whole i-slices).
  - Host gathers xg[j,p,k] = x[j, ker[p,k]] (small) into two resident SBUF
    tiles (main 128 rows = (j, k<8); leftover 128 rows = (i%8, j) for k=8).
  - Device: per chunk, one DVE elementwise multiply mask_chunk * xgather tile,
    then the TensorEngine contracts the 128-partition axis with a host-built
    block-selector weight matrix that has W folded in (psum[i,p] accumulates
    over all 18 chunks).
  - ScalarE applies bias + exact ELU:  elu(v) = relu(v) + exp(-relu(-v)) - 1.
  - When the mask is exactly representable in bf16 (e.g. binary 0/1 masks),
    it is shipped as bf16: halves the dominant DMA with bit-exact results.
"""

import os

import numpy as np

IN, OUT, KF = 16, 16, 9
N = 16384
NCORES = 8
PC = N // NCORES  # positions per core
NCHUNK = 18  # 16 main chunks (one per i) + 2 leftover chunks (k=8 taps)
PBLK = 512  # fp32 matmul free-dim / one PSUM bank
NB = PC // PBLK

# Chunks whose elementwise multiply runs on GPSIMD instead of DVE (load balance).
GPSIMD_CHUNKS = frozenset()

_CACHE: dict = {}


def _build(mask_is_bf16: bool):
    import concourse.bacc as bacc
    import concourse.mybir as mybir
    from concourse.tile import TileContext

    f32 = mybir.dt.float32
    mdt = mybir.dt.bfloat16 if mask_is_bf16 else f32
    AF = mybir.ActivationFunctionType

    nc = bacc.Bacc("TRN2", target_bir_lowering=False, debug=False)

    mt = nc.dram_tensor("mt", (NCHUNK * 128, PC), mdt, kind="ExternalInput")
    xm = nc.dram_tensor("xm", (128, PC), f32, kind="ExternalInput")
    x9 = nc.dram_tensor("x9", (128, PC), f32, kind="ExternalInput")
    wseg = nc.dram_tensor("wseg", (128, NCHUNK * OUT), f32, kind="ExternalInput")
    bpm = nc.dram_tensor("bpm", (OUT, 2), f32, kind="ExternalInput")
    out = nc.dram_tensor("out", (OUT, PC), f32, kind="ExternalOutput")

    with TileContext(nc) as tc:
        with (
            tc.tile_pool(name="const", bufs=1) as cpool,
            tc.tile_pool(name="mask", bufs=4) as mpool,
            tc.tile_pool(name="prod", bufs=3) as ppool,
            tc.tile_pool(name="elu", bufs=2) as tpool,
            tc.tile_pool(name="outp", bufs=1) as opool,
            tc.tile_pool(name="psum", bufs=1, space="PSUM") as qpool,
        ):
            xm_t = cpool.tile([128, PC], f32, tag="xm")
            nc.sync.dma_start(xm_t, xm[:, :])
            x9_t = cpool.tile([128, PC], f32, tag="x9")
            nc.sync.dma_start(x9_t, x9[:, :])
            w_t = cpool.tile([128, NCHUNK * OUT], f32, tag="w")
            nc.sync.dma_start(w_t, wseg[:, :])
            b_t = cpool.tile([OUT, 2], f32, tag="b")
            nc.sync.dma_start(b_t, bpm[:, :])

            psums = [
                qpool.tile([OUT, PBLK], f32, tag=f"ps{bi}", name=f"ps{bi}")
                for bi in range(NB)
            ]

            for c in range(NCHUNK):
                m_t = mpool.tile([128, PC], mdt, tag="m")
                nc.sync.dma_start(m_t, mt[c * 128 : (c + 1) * 128, :])
                p_t = ppool.tile([128, PC], f32, tag="p")
                src = xm_t if c < 16 else x9_t
                eng = nc.gpsimd if c in GPSIMD_CHUNKS else nc.vector
                eng.tensor_mul(p_t, m_t, src)
                for bi in range(NB):
                    nc.tensor.matmul(
                        psums[bi],
                        w_t[:, c * OUT : (c + 1) * OUT],
                        p_t[:, bi * PBLK : (bi + 1) * PBLK],
                        start=(c == 0),
                        stop=(c == NCHUNK - 1),
                    )

            out_t = opool.tile([OUT, PC], f32, tag="o")
            for bi in range(NB):
                ps = psums[bi]
                r_t = tpool.tile([OUT, PBLK], f32, tag="r")
                # r = relu(y + b)
                nc.scalar.activation(r_t, ps, AF.Relu, bias=b_t[:, 0:1], scale=1.0)
                n_t = tpool.tile([OUT, PBLK], f32, tag="n")
                # n = relu(-(y + b)) = -min(y + b, 0)
                nc.scalar.activation(n_t, ps, AF.Relu, bias=b_t[:, 1:2], scale=-1.0)
                e_t = tpool.tile([OUT, PBLK], f32, tag="e")
                # e = exp(-n) = exp(min(y + b, 0))
                nc.scalar.activation(e_t, n_t, AF.Exp, scale=-1.0)
                nc.vector.tensor_add(out_t[:, bi * PBLK : (bi + 1) * PBLK], r_t, e_t)
            # elu(v) = relu(v) + exp(min(v, 0)) - 1
            nc.vector.tensor_scalar_add(out_t, out_t, -1.0)
            nc.sync.dma_start(out[:, :], out_t)

    nc.compile()
    return nc


def get_kernel(mask_is_bf16: bool):
    key = ("bf16" if mask_is_bf16 else "f32",)
    if key not in _CACHE:
        _CACHE[key] = _build(mask_is_bf16)
    return _CACHE[key]


def prepare_inputs(x, W, b, mask, ker, force_f32=False):
    """Host-side sharding/layout prep. Returns (in_maps, mask_is_bf16)."""
    import ml_dtypes

    x = np.ascontiguousarray(np.asarray(x, np.float32))
    W = np.asarray(W, np.float32)
    b = np.asarray(b, np.float32)
    mask = np.asarray(mask, np.float32)
    ker = np.asarray(ker)

    # xg[j, p, k] = x[j, ker[p, k]]
    xg = x[:, ker]  # (16, N, 9)
    xmain = np.ascontiguousarray(
        xg[:, :, :8].transpose(0, 2, 1).reshape(128, N)
    )  # row j*8+k
    x9r = np.ascontiguousarray(np.tile(xg[:, :, 8], (8, 1)))  # row q -> j=q%16

    # bf16 fast path iff exact (e.g. binary masks)
    mask_is_bf16 = False
    mcast = mask
    if not force_f32:
        mb = mask.astype(ml_dtypes.bfloat16)
        if np.array_equal(mb.astype(np.float32), mask):
            mask_is_bf16 = True
            mcast = mb

    # Block-selector weights with W folded in: [128, NCHUNK*OUT]
    wsegm = np.zeros((128, NCHUNK * OUT), np.float32)
    wmain = W[:, :, :8].reshape(16, 128)  # [i, j*8+k]
    for i in range(16):
        wsegm[:, i * OUT + i] = wmain[i]
    w8 = W[:, :, 8]  # [i, j]
    for di in range(8):
        wsegm[di * 16 : (di + 1) * 16, 16 * OUT + di] = w8[di]
        wsegm[di * 16 : (di + 1) * 16, 17 * OUT + 8 + di] = w8[8 + di]
    bpm = np.ascontiguousarray(np.stack([b, -b], axis=1).astype(np.float32))

    in_maps = []
    for ci in range(NCORES):
        sl = slice(ci * PC, (ci + 1) * PC)
        msl = mcast[sl]  # (PC, 16, 16, 9)
        mm = msl[:, :, :, :8].transpose(1, 2, 3, 0).reshape(2048, PC)
        mlast = msl[:, :, :, 8].transpose(1, 2, 0).reshape(256, PC)
        mt_core = np.ascontiguousarray(np.concatenate([mm, mlast], axis=0))
        in_maps.append(
            {
                "mt": mt_core,
                "xm": np.ascontiguousarray(xmain[:, sl]),
                "x9": np.ascontiguousarray(x9r[:, sl]),
                "wseg": wsegm,
                "bpm": bpm,
            }
        )
    return in_maps, mask_is_bf16


def kernel(x, W, b, mask, ker, _trace=False):
    from concourse import bass_utils

    in_maps, mask_is_bf16 = prepare_inputs(
        x, W, b, mask, ker, force_f32=bool(int(os.environ.get("KERNEL_FORCE_F32", "0")))
    )
    nc = get_kernel(mask_is_bf16)
    res = bass_utils.run_bass_kernel_spmd(
        nc, in_maps, core_ids=list(range(NCORES)), trace=_trace
    )
    outs = [res.results[ci]["out"] for ci in range(NCORES)]
    full = np.concatenate(outs, axis=1).astype(np.float32)
    if _trace:
        return full, res
    return full


# revision 11
# speedup vs baseline: 1.1862x; 1.1862x over previous
"""Trainium2 Bass kernel for the masked per-position CNN contraction.

Computes, for each lattice position p (N=16384 total):
    y[p, i] = sum_{j,k} W[i,j,k] * mask[p,i,j,k] * x[j, ker[p,k]]
    out[i, p] = elu(y[p, i] + b[i])

Strategy (8 NeuronCores, position axis sharded 2048/core):
  - mask (151 MB fp32) dominates traffic -> memory-bound. Host transposes the
    per-core mask slice to a [(i,j,k), p] layout whose 2304-long contraction
    axis is split into 18 partition-aligned chunks of 128 (k=9 is split 8+1 so
    chunk boundaries align with whole i-slices).
  - Host gathers xg[j,p,k] = x[j, ker[p,k]] (small) into two resident SBUF
    tiles (main 128 rows = (j, k<8); leftover 128 rows = (i%8, j) for k=8).
  - Device: per chunk, one DVE elementwise multiply mask_chunk * xgather tile,
    then the TensorEngine contracts the 128-partition axis with a host-built
    block-selector weight matrix that has W folded in (psum[i,p] accumulates
    over all 18 chunks).
  - ScalarE applies bias + exact ELU:  elu(v) = relu(v) + exp(-relu(-v)) - 1.
  - When the mask is exactly representable in bf16 (e.g. binary 0/1 masks),
    it is shipped as bf16: halves the dominant DMA with bit-exact results.
"""

import os

import numpy as np

IN, OUT, KF = 16, 16, 9
N = 16384
NCORES = 8
PC = N // NCORES  # positions per core
NCHUNK = 18  # 16 main chunks (one per i) + 2 leftover chunks (k=8 taps)
PBLK = 512  # fp32 matmul free-dim / one PSUM bank
NB = PC // PBLK

# Chunks whose elementwise multiply runs on GPSIMD instead of DVE (load balance).
GPSIMD_CHUNKS = frozenset({3, 7, 11, 15})

_CACHE: dict = {}


def _build(mask_is_bf16: bool):
    import concourse.bacc as bacc
    import concourse.mybir as mybir
    from concourse.tile import TileContext

    f32 = mybir.dt.float32
    f32r = mybir.dt.float32r
    mdt = mybir.dt.bfloat16 if mask_is_bf16 else f32
    AF = mybir.ActivationFunctionType

    nc = bacc.Bacc("TRN2", target_bir_lowering=False, debug=False)

    mt = nc.dram_tensor("mt", (NCHUNK * 128, PC), mdt, kind="ExternalInput")
    xm = nc.dram_tensor("xm", (128, PC), f32, kind="ExternalInput")
    x9 = nc.dram_tensor("x9", (128, PC), f32, kind="ExternalInput")
    wseg = nc.dram_tensor("wseg", (128, NCHUNK * OUT), f32r, kind="ExternalInput")
    bpm = nc.dram_tensor("bpm", (OUT, 2), f32, kind="ExternalInput")
    out = nc.dram_tensor("out", (OUT, PC), f32, kind="ExternalOutput")

    with TileContext(nc) as tc:
        with (
            tc.tile_pool(name="const", bufs=1) as cpool,
            tc.tile_pool(name="mask", bufs=4) as mpool,
            tc.tile_pool(name="prod", bufs=3) as ppool,
            tc.tile_pool(name="elu", bufs=2) as tpool,
            tc.tile_pool(name="outp", bufs=1) as opool,
            tc.tile_pool(name="psum", bufs=1, space="PSUM") as qpool,
        ):
            xm_t = cpool.tile([128, PC], f32, tag="xm")
            nc.sync.dma_start(xm_t, xm[:, :])
            x9_t = cpool.tile([128, PC], f32, tag="x9")
            nc.sync.dma_start(x9_t, x9[:, :])
            w_t = cpool.tile([128, NCHUNK * OUT], f32r, tag="w")
            nc.sync.dma_start(w_t, wseg[:, :])
            b_t = cpool.tile([OUT, 2], f32, tag="b")
            nc.sync.dma_start(b_t, bpm[:, :])

            psums = [
                qpool.tile([OUT, PBLK], f32, tag=f"ps{bi}", name=f"ps{bi}")
                for bi in range(NB)
            ]

            for c in range(NCHUNK):
                m_t = mpool.tile([128, PC], mdt, tag="m")
                nc.sync.dma_start(m_t, mt[c * 128 : (c + 1) * 128, :])
                p_t = ppool.tile([128, PC], f32r, tag="p")
                src = xm_t if c < 16 else x9_t
                eng = nc.gpsimd if c in GPSIMD_CHUNKS else nc.vector
                eng.tensor_mul(p_t, m_t, src)
                for bi in range(NB):
                    # float32r: full fp32 data layout, single-pass matmul at
                    # bf16 rate (vs fp32's 2 half-rate passes + dual LDWEIGHTS)
                    nc.tensor.matmul(
                        psums[bi],
                        w_t[:, c * OUT : (c + 1) * OUT],
                        p_t[:, bi * PBLK : (bi + 1) * PBLK],
                        start=(c == 0),
                        stop=(c == NCHUNK - 1),
                    )

            out_t = opool.tile([OUT, PC], f32, tag="o")
            for bi in range(NB):
                ps = psums[bi]
                r_t = tpool.tile([OUT, PBLK], f32, tag="r")
                # r = relu(y + b)
                nc.scalar.activation(r_t, ps, AF.Relu, bias=b_t[:, 0:1], scale=1.0)
                n_t = tpool.tile([OUT, PBLK], f32, tag="n")
                # n = relu(-(y + b)) = -min(y + b, 0)
                nc.scalar.activation(n_t, ps, AF.Relu, bias=b_t[:, 1:2], scale=-1.0)
                e_t = tpool.tile([OUT, PBLK], f32, tag="e")
                # e = exp(-n) = exp(min(y + b, 0))
                nc.scalar.activation(e_t, n_t, AF.Exp, scale=-1.0)
                # elu(v) = relu(v) + exp(min(v, 0)) - 1 = (r - 1) + e
                nc.vector.scalar_tensor_tensor(
                    out_t[:, bi * PBLK : (bi + 1) * PBLK],
                    r_t,
                    -1.0,
                    e_t,
                    mybir.AluOpType.add,
                    mybir.AluOpType.add,
                )
            nc.sync.dma_start(out[:, :], out_t)

    nc.compile()
    return nc


def get_kernel(mask_is_bf16: bool):
    key = ("bf16" if mask_is_bf16 else "f32",)
    if key not in _CACHE:
        _CACHE[key] = _build(mask_is_bf16)
    return _CACHE[key]


def prepare_inputs(x, W, b, mask, ker, force_f32=False):
    """Host-side sharding/layout prep. Returns (in_maps, mask_is_bf16)."""
    import ml_dtypes

    x = np.ascontiguousarray(np.asarray(x, np.float32))
    W = np.asarray(W, np.float32)
    b = np.asarray(b, np.float32)
    mask = np.asarray(mask, np.float32)
    ker = np.asarray(ker)

    # xg[j, p, k] = x[j, ker[p, k]]
    xg = x[:, ker]  # (16, N, 9)
    xmain = np.ascontiguousarray(
        xg[:, :, :8].transpose(0, 2, 1).reshape(128, N)
    )  # row j*8+k
    x9r = np.ascontiguousarray(np.tile(xg[:, :, 8], (8, 1)))  # row q -> j=q%16

    # bf16 fast path iff exact (e.g. binary masks)
    mask_is_bf16 = False
    mcast = mask
    if not force_f32:
        mb = mask.astype(ml_dtypes.bfloat16)
        if np.array_equal(mb.astype(np.float32), mask):
            mask_is_bf16 = True
            mcast = mb

    # Block-selector weights with W folded in: [128, NCHUNK*OUT]
    wsegm = np.zeros((128, NCHUNK * OUT), np.float32)
    wmain = W[:, :, :8].reshape(16, 128)  # [i, j*8+k]
    for i in range(16):
        wsegm[:, i * OUT + i] = wmain[i]
    w8 = W[:, :, 8]  # [i, j]
    for di in range(8):
        wsegm[di * 16 : (di + 1) * 16, 16 * OUT + di] = w8[di]
        wsegm[di * 16 : (di + 1) * 16, 17 * OUT + 8 + di] = w8[8 + di]
    bpm = np.ascontiguousarray(np.stack([b, -b], axis=1).astype(np.float32))

    in_maps = []
    for ci in range(NCORES):
        sl = slice(ci * PC, (ci + 1) * PC)
        msl = mcast[sl]  # (PC, 16, 16, 9)
        mm = msl[:, :, :, :8].transpose(1, 2, 3, 0).reshape(2048, PC)
        mlast = msl[:, :, :, 8].transpose(1, 2, 0).reshape(256, PC)
        mt_core = np.ascontiguousarray(np.concatenate([mm, mlast], axis=0))
        in_maps.append(
            {
                "mt": mt_core,
                "xm": np.ascontiguousarray(xmain[:, sl]),
                "x9": np.ascontiguousarray(x9r[:, sl]),
                "wseg": wsegm,
                "bpm": bpm,
            }
        )
    return in_maps, mask_is_bf16


def kernel(x, W, b, mask, ker, _trace=False):
    from concourse import bass_utils

    in_maps, mask_is_bf16 = prepare_inputs(
        x, W, b, mask, ker, force_f32=bool(int(os.environ.get("KERNEL_FORCE_F32", "0")))
    )
    nc = get_kernel(mask_is_bf16)
    res = bass_utils.run_bass_kernel_spmd(
        nc, in_maps, core_ids=list(range(NCORES)), trace=_trace
    )
    outs = [res.results[ci]["out"] for ci in range(NCORES)]
    full = np.concatenate(outs, axis=1).astype(np.float32)
    if _trace:
        return full, res
    return full


# revision 18
# speedup vs baseline: 1.4780x; 1.2460x over previous
"""Trainium2 Bass kernel for the masked per-position CNN contraction.

Computes, for each lattice position p (N=16384 total):
    y[p, i] = sum_{j,k} W[i,j,k] * mask[p,i,j,k] * x[j, ker[p,k]]
    out[i, p] = elu(y[p, i] + b[i])

Strategy (8 NeuronCores, position axis sharded 2048/core):
  - mask (151 MB fp32) dominates traffic -> memory-bound. Host transposes the
    per-core mask slice to a [(i,j,k), p] layout whose 2304-long contraction
    axis is split into 18 partition-aligned chunks of 128 (k=9 is split 8+1 so
    chunk boundaries align with whole i-slices).
  - Host gathers xg[j,p,k] = x[j, ker[p,k]] (small) into two resident SBUF
    tiles (main 128 rows = (j, k<8); leftover 128 rows = (i%8, j) for k=8).
  - Device: per chunk, one DVE elementwise multiply mask_chunk * xgather tile,
    then the TensorEngine contracts the 128-partition axis with a host-built
    block-selector weight matrix that has W folded in (psum[i,p] accumulates
    over all 18 chunks).
  - ScalarE applies bias + exact ELU:  elu(v) = relu(v) + exp(-relu(-v)) - 1.
  - When the mask is exactly representable in bf16 (e.g. binary 0/1 masks),
    it is shipped as bf16: halves the dominant DMA with bit-exact results.
"""

import os

import numpy as np

IN, OUT, KF = 16, 16, 9
N = 16384
NCORES = 8
PC = N // NCORES  # positions per core
NCHUNK = 18  # 16 main chunks (one per i) + 2 leftover chunks (k=8 taps)
PBLK = 512  # fp32 matmul free-dim / one PSUM bank
NB = PC // PBLK

# Chunks whose elementwise multiply runs on GPSIMD instead of DVE (load balance).
GPSIMD_CHUNKS = frozenset()

_CACHE: dict = {}


def _build(mask_is_bf16: bool):
    import concourse.bacc as bacc
    import concourse.mybir as mybir
    from concourse.tile import TileContext

    f32 = mybir.dt.float32
    f32r = mybir.dt.float32r
    mdt = mybir.dt.bfloat16 if mask_is_bf16 else f32
    AF = mybir.ActivationFunctionType

    nc = bacc.Bacc("TRN2", target_bir_lowering=False, debug=False)

    mt = nc.dram_tensor("mt", (NCHUNK * 128, PC), mdt, kind="ExternalInput")
    xm = nc.dram_tensor("xm", (128, PC), f32, kind="ExternalInput")
    x9 = nc.dram_tensor("x9", (128, PC), f32, kind="ExternalInput")
    wseg = nc.dram_tensor("wseg", (128, NCHUNK * OUT), f32r, kind="ExternalInput")
    bpm = nc.dram_tensor("bpm", (OUT, 2), f32, kind="ExternalInput")
    out = nc.dram_tensor("out", (OUT, PC), f32, kind="ExternalOutput")

    with TileContext(nc) as tc:
        with (
            tc.tile_pool(name="const", bufs=1) as cpool,
            tc.tile_pool(name="mask", bufs=6) as mpool,
            tc.tile_pool(name="prod", bufs=3) as ppool,
            tc.tile_pool(name="elu", bufs=2) as tpool,
            tc.tile_pool(name="psum", bufs=1, space="PSUM") as qpool,
        ):
            # Constants ride the ScalarE HWDGE ring so the Sync ring is
            # dedicated to the mask stream.
            xm_t = cpool.tile([128, PC], f32, tag="xm")
            nc.scalar.dma_start(xm_t, xm[:, :])
            x9_t = cpool.tile([128, PC], f32, tag="x9")
            nc.scalar.dma_start(x9_t, x9[:, :])
            w_t = cpool.tile([128, NCHUNK * OUT], f32r, tag="w")
            nc.scalar.dma_start(w_t, wseg[:, :])
            b_t = cpool.tile([OUT, 2], f32, tag="b")
            nc.scalar.dma_start(b_t, bpm[:, :])

            psums = [
                qpool.tile([OUT, PBLK], f32, tag=f"ps{bi}", name=f"ps{bi}")
                for bi in range(NB)
            ]

            for c in range(NCHUNK):
                m_t = mpool.tile([128, PC], mdt, tag="m")
                nc.sync.dma_start(m_t, mt[c * 128 : (c + 1) * 128, :])
                p_t = ppool.tile([128, PC], f32r, tag="p")
                src = xm_t if c < 16 else x9_t
                nc.vector.tensor_mul(p_t, m_t, src)
                for bi in range(NB):
                    # float32r: full fp32 data layout, single-pass matmul
                    # (vs fp32's 2 half-rate passes + dual LDWEIGHTS)
                    nc.tensor.matmul(
                        psums[bi],
                        w_t[:, c * OUT : (c + 1) * OUT],
                        p_t[:, bi * PBLK : (bi + 1) * PBLK],
                        start=(c == 0),
                        stop=(c == NCHUNK - 1),
                    )

            # elu(v) = relu(v) + exp(min(v, 0)) - 1, split across ACT + DVE
            for bi in range(NB):
                ps = psums[bi]
                r_t = tpool.tile([OUT, PBLK], f32, tag="r")
                # ACT: r = relu(y + b)
                nc.scalar.activation(r_t, ps, AF.Relu, bias=b_t[:, 0:1], scale=1.0)
                mn_t = tpool.tile([OUT, PBLK], f32, tag="mn")
                # DVE: mn = min(y + b, 0)
                nc.vector.tensor_scalar(
                    mn_t, ps, b_t[:, 0:1], 0.0,
                    mybir.AluOpType.add, mybir.AluOpType.min,
                )
                e_t = tpool.tile([OUT, PBLK], f32, tag="e")
                # ACT: e = exp(mn)
                nc.scalar.activation(e_t, mn_t, AF.Exp)
                o_t = tpool.tile([OUT, PBLK], f32, tag="o")
                # DVE: out = (r - 1) + e
                nc.vector.scalar_tensor_tensor(
                    o_t, r_t, -1.0, e_t,
                    mybir.AluOpType.add, mybir.AluOpType.add,
                )
                nc.sync.dma_start(out[:, bi * PBLK : (bi + 1) * PBLK], o_t)

    nc.compile()
    return nc


def get_kernel(mask_is_bf16: bool):
    key = ("bf16" if mask_is_bf16 else "f32",)
    if key not in _CACHE:
        _CACHE[key] = _build(mask_is_bf16)
    return _CACHE[key]


def prepare_inputs(x, W, b, mask, ker, force_f32=False):
    """Host-side sharding/layout prep. Returns (in_maps, mask_is_bf16)."""
    import ml_dtypes

    x = np.ascontiguousarray(np.asarray(x, np.float32))
    W = np.asarray(W, np.float32)
    b = np.asarray(b, np.float32)
    mask = np.asarray(mask, np.float32)
    ker = np.asarray(ker)

    # xg[j, p, k] = x[j, ker[p, k]]
    xg = x[:, ker]  # (16, N, 9)
    xmain = np.ascontiguousarray(
        xg[:, :, :8].transpose(0, 2, 1).reshape(128, N)
    )  # row j*8+k
    x9r = np.ascontiguousarray(np.tile(xg[:, :, 8], (8, 1)))  # row q -> j=q%16

    # bf16 fast path iff exact (e.g. binary masks)
    mask_is_bf16 = False
    mcast = mask
    if not force_f32:
        mb = mask.astype(ml_dtypes.bfloat16)
        if np.array_equal(mb.astype(np.float32), mask):
            mask_is_bf16 = True
            mcast = mb

    # Block-selector weights with W folded in: [128, NCHUNK*OUT]
    wsegm = np.zeros((128, NCHUNK * OUT), np.float32)
    wmain = W[:, :, :8].reshape(16, 128)  # [i, j*8+k]
    for i in range(16):
        wsegm[:, i * OUT + i] = wmain[i]
    w8 = W[:, :, 8]  # [i, j]
    for di in range(8):
        wsegm[di * 16 : (di + 1) * 16, 16 * OUT + di] = w8[di]
        wsegm[di * 16 : (di + 1) * 16, 17 * OUT + 8 + di] = w8[8 + di]
    bpm = np.ascontiguousarray(np.stack([b, -b], axis=1).astype(np.float32))

    in_maps = []
    for ci in range(NCORES):
        sl = slice(ci * PC, (ci + 1) * PC)
        msl = mcast[sl]  # (PC, 16, 16, 9)
        mm = msl[:, :, :, :8].transpose(1, 2, 3, 0).reshape(2048, PC)
        mlast = msl[:, :, :, 8].transpose(1, 2, 0).reshape(256, PC)
        mt_core = np.ascontiguousarray(np.concatenate([mm, mlast], axis=0))
        in_maps.append(
            {
                "mt": mt_core,
                "xm": np.ascontiguousarray(xmain[:, sl]),
                "x9": np.ascontiguousarray(x9r[:, sl]),
                "wseg": wsegm,
                "bpm": bpm,
            }
        )
    return in_maps, mask_is_bf16


def kernel(x, W, b, mask, ker, _trace=False):
    from concourse import bass_utils

    in_maps, mask_is_bf16 = prepare_inputs(
        x, W, b, mask, ker, force_f32=bool(int(os.environ.get("KERNEL_FORCE_F32", "0")))
    )
    nc = get_kernel(mask_is_bf16)
    res = bass_utils.run_bass_kernel_spmd(
        nc, in_maps, core_ids=list(range(NCORES)), trace=_trace
    )
    outs = [res.results[ci]["out"] for ci in range(NCORES)]
    full = np.concatenate(outs, axis=1).astype(np.float32)
    if _trace:
        return full, res
    return full
